# revision 1
# baseline (speedup 1.0000x reference)
import sys
sys.path.insert(0, "/opt/trn_rl_repo")
import numpy as np
import concourse.bass as bass
import concourse.mybir as mybir
from concourse import tile
from concourse.bass_utils import run_bass_kernel_spmd
import bass_rust as _br

NC_N = 8
B, S, F = 4096, 41, 128
BL = B // NC_N
Q, BQ = 4, 128
F32 = mybir.dt.float32
F32R = mybir.dt.float32r
F16 = mybir.dt.float16
AF = mybir.ActivationFunctionType
OP = mybir.AluOpType
AX = mybir.AxisListType
EPS_BN = 1e-5
EPS_SQ = 1e-21

NBN0 = float(B * 128)
NBN1 = float(B * 64)
NBN2 = float(B * 32)
NT = 2 * B


def fix_instruction_waits(nc):
    cnt = 0
    for bb in nc.main_func.blocks:
        out = []
        for ins in bb.instructions:
            si = ins.sync_info
            if si is not None and len(si.on_wait) > 1:
                waits = list(si.on_wait)
                for w in waits[:-1]:
                    nop = _br.InstNoOp(name=f"wsplit{cnt}", ins=[], outs=[])
                    cnt += 1
                    nop.engine = ins.engine
                    nop.sync_info = _br.SyncInfo(on_wait=[w], on_update=[])
                    out.append(nop)
                ins.sync_info = _br.SyncInfo(on_wait=[waits[-1]],
                                             on_update=list(si.on_update))
            out.append(ins)
        bb.instructions[:] = out
    return cnt


def build(dbg=None, stop=7):
    nc = bass.Bass()
    P = nc.declare_dram_parameter
    oh_d = P("oh", [4, S + 2, BL], F32R, isOutput=False)
    w3_d = P("w3", [S, 4, 3, F], F32R, isOutput=False)
    w5_d = P("w5", [S, F, 5, F], F32R, isOutput=False)
    w7_d = P("w7", [S, F, 7, F], F32R, isOutput=False)
    cw1_d = P("cw1", [F, 64], F32R, isOutput=False)
    cwc_d = P("cwc", [64, 3, 64], F32R, isOutput=False)
    rwt_d = P("rwt", [64, F, 2], F32R, isOutput=False)
    fc1_d = P("fc1", [F, 64], F32R, isOutput=False)
    fc2_d = P("fc2", [64, 32], F32R, isOutput=False)
    fc3_d = P("fc3", [32, 1], F32R, isOutput=False)
    vecs_d = P("vecs", [F, 128], F32, isOutput=False)
    vrow_d = P("vrow", [1, 80], F32, isOutput=False)
    vones_d = P("vones", [1, F], F32R, isOutput=False)
    ones8_d = P("ones8", [F, 8], F32R, isOutput=False)
    ident_d = P("ident", [F, F], F32, isOutput=False)
    out_d = P("out", [B], F32, isOutput=True)
    dbg_d = None
    if dbg in ("h1", "h2", "h3"):
        dbg_d = P("dbg", [F, S, 16], F32, isOutput=True)
    elif dbg in ("y", "z", "yp"):
        dbg_d = P("dbg", [64, 16, F], F32, isOutput=True)
    elif dbg == "s":
        dbg_d = P("dbg", [2, F, 16], F32, isOutput=True)
    elif dbg == "bn":
        dbg_d = P("dbg", [64, 4], F32, isOutput=True)
    elif dbg == "y1":
        dbg_d = P("dbg", [64, 16], F32, isOutput=True)

    with tile.TileContext(nc) as tc:
        with (
            tc.tile_pool(name="pC", bufs=1) as pC,
            tc.tile_pool(name="pSm", bufs=1) as pSm,
            tc.tile_pool(name="pp", bufs=2, space="PSUM") as pp,
            tc.tile_pool(name="pt", bufs=2, space="PSUM") as pt,
            tc.tile_pool(name="pr", bufs=2, space="PSUM") as pr,
            tc.tile_pool(name="dram", bufs=1, space="DRAM") as dram,
        ):
            cw1 = pC.tile([F, 64], F32R, tag="cw1")
            nc.sync.dma_start(cw1[:], cw1_d[:])
            cwc = pC.tile([64, 3, 64], F32R, tag="cwc")
            nc.sync.dma_start(cwc[:], cwc_d[:])
            rwt = pC.tile([64, F, 2], F32R, tag="rwt")
            nc.sync.dma_start(rwt[:], rwt_d[:])
            fc1 = pC.tile([F, 64], F32R, tag="fc1")
            nc.sync.dma_start(fc1[:], fc1_d[:])
            fc2 = pC.tile([64, 32], F32R, tag="fc2")
            nc.sync.dma_start(fc2[:], fc2_d[:])
            fc3 = pC.tile([32, 1], F32R, tag="fc3")
            nc.sync.dma_start(fc3[:], fc3_d[:])
            vecs = pC.tile([F, 128], F32, tag="vecs")
            nc.sync.dma_start(vecs[:], vecs_d[:])
            vrow = pC.tile([1, 80], F32, tag="vrow")
            nc.sync.dma_start(vrow[:], vrow_d[:])
            vones = pC.tile([1, F], F32R, tag="vones")
            nc.sync.dma_start(vones[:], vones_d[:])
            ones8 = pC.tile([F, 8], F32R, tag="ones8")
            nc.sync.dma_start(ones8[:], ones8_d[:])
            ident = pC.tile([F, F], F32, tag="ident")
            nc.sync.dma_start(ident[:], ident_d[:])

            y_dram = dram.tile([64, BL, F], F16)
            st1_in = dram.tile([F, 2], F32)
            st1_out = dram.tile([F, 2], F32)
            s_loc = dram.tile([2, F, BL], F32)
            s_all = dram.tile([2 * NC_N, F, BL], F32)
            bscr = dram.tile([1, 8], F32)

            def relu_copy(i, dst, psum, bias_ap):
                if i % 2 == 0:
                    nc.scalar.activation(dst, psum, AF.Relu, bias=bias_ap)
                else:
                    nc.vector.tensor_scalar(dst, psum, bias_ap, 0.0, OP.add, OP.max)

            c1s = pSm.tile([64, 128], F32, tag="c1s")
            c1q = pSm.tile([64, 128], F32, tag="c1q")
            mut = pSm.tile([64, 4], F32, tag="mut")

            with tc.tile_pool(name="pH", bufs=1) as pH:
                with tc.tile_pool(name="pW", bufs=2) as pW, \
                     tc.tile_pool(name="pOh", bufs=5) as pOh:
                    h1 = pH.tile([F, S, BL], F32R, tag="A")
                    ohts = {}
                    for p in (0, 1):
                        ohts[p] = pOh.tile([4, BL], F32R, tag="oh", name=f"oht{p}")
                        nc.sync.dma_start(ohts[p][:], oh_d[:, p, :])
                    for s in range(S):
                        p = s + 2
                        ohts[p] = pOh.tile([4, BL], F32R, tag="oh", name=f"oht{p}")
                        nc.sync.dma_start(ohts[p][:], oh_d[:, p, :])
                        w3t = pW.tile([4, 3, F], F32R, tag="w3")
                        nc.sync.dma_start(w3t[:], w3_d[s])
                        ps1 = pp.tile([F, BL], F32, tag="mm")
                        for j in range(3):
                            nc.tensor.matmul(ps1[:], w3t[:, j, :], ohts[s + j][:],
                                             start=(j == 0), stop=(j == 2))
                        relu_copy(s, h1[:, s, :], ps1[:], vecs[:, s:s + 1])

                    h2 = pH.tile([F, S, BL], F32R, tag="B")
                    for s in range(S):
                        w5t = pW.tile([F, 5, F], F32R, tag="w5")
                        nc.sync.dma_start(w5t[:], w5_d[s])
                        ps2 = pp.tile([F, BL], F32, tag="mm")
                        taps = [j for j in range(5) if 0 <= s + j - 2 < S]
                        for n, j in enumerate(taps):
                            nc.tensor.matmul(ps2[:], w5t[:, j, :], h1[:, s + j - 2, :],
                                             start=(n == 0), stop=(n == len(taps) - 1))
                        relu_copy(s, h2[:, s, :], ps2[:], vecs[:, 41 + s:42 + s])

                    h3 = pH.tile([F, S, BL], F32R, tag="A")
                    for s in range(S):
                        w7t = pW.tile([F, 7, F], F32R, tag="w7")
                        nc.sync.dma_start(w7t[:], w7_d[s])
                        ps3 = pp.tile([F, BL], F32, tag="mm")
                        taps = [j for j in range(7) if 0 <= s + j - 3 < S]
                        for n, j in enumerate(taps):
                            nc.tensor.matmul(ps3[:], w7t[:, j, :], h2[:, s + j - 3, :],
                                             start=(n == 0), stop=(n == len(taps) - 1))
                        relu_copy(s, h3[:, s, :], ps3[:], vecs[:, 82 + s:83 + s])

                    if dbg == "h1":
                        nc.sync.dma_start(dbg_d[:], h1[:, :, 0:16].bitcast(F32))
                    if dbg == "h2":
                        nc.sync.dma_start(dbg_d[:], h2[:, :, 0:16].bitcast(F32))
                    if dbg == "h3":
                        nc.sync.dma_start(dbg_d[:], h3[:, :, 0:16].bitcast(F32))

                with tc.tile_pool(name="pStg", bufs=3) as pStg, \
                     tc.tile_pool(name="pX", bufs=2) as pX:
                    for q in range(Q if stop >= 4 else 0):
                        t123 = pH.tile([F, BQ, F], F32R, tag="B")
                        nc.vector.memset(t123[0:64, :, 0:1].bitcast(F32), 0.0)
                        nc.vector.memset(t123[64:128, :, 127:128].bitcast(F32), 0.0)
                        for c in range(S):
                            ptp = pt.tile([F, F], F32, tag="tp")
                            nc.tensor.transpose(
                                ptp[:], h3[:, c, q * BQ:(q + 1) * BQ].bitcast(F32),
                                ident[:])
                            tstg = pStg.tile([F, F], F32, tag="tstg")
                            if c % 2 == 0:
                                nc.vector.tensor_copy(tstg[:], ptp[:])
                            else:
                                nc.scalar.copy(tstg[:], ptp[:])
                            nc.sync.dma_start(t123[c:c + 1, :, 1:128],
                                              tstg[:, 0:127].bitcast(F32R))
                            nc.sync.dma_start(t123[41 + c:42 + c, :, 0:128],
                                              tstg[:, 0:128].bitcast(F32R))
                            nc.sync.dma_start(t123[82 + c:83 + c, :, 0:127],
                                              tstg[:, 1:128].bitcast(F32R))
                        for i in range(32):
                            pcy = pp.tile([F, 512], F32, tag="mm")
                            nc.tensor.matmul(
                                pcy[0:64, :].rearrange("p (b l) -> p b l", b=4),
                                cw1[:, :], t123[:, 4 * i:4 * i + 4, :],
                                start=True, stop=True)
                            ci = 32 * q + i
                            ystg = pStg.tile([64, 512], F16, tag="ystg")
                            nc.scalar.activation(ystg[:], pcy[0:64, :], AF.Identity,
                                                 bias=vecs[0:64, 123:124],
                                                 accum_out=c1s[:, ci:ci + 1])
                            junk = pStg.tile([64, 512], F16, tag="junk")
                            nc.vector.scalar_tensor_tensor(
                                junk[:], ystg[:], 1.0, ystg[:], OP.mult, OP.mult,
                                accum_out=c1q[:, ci:ci + 1])
                            b0 = q * BQ + 4 * i
                            nc.sync.dma_start(
                                y_dram[:, b0:b0 + 4, :],
                                ystg[:].rearrange("p (b l) -> p b l", b=4))

                    if stop >= 4:
                        st1 = pSm.tile([64, 2], F32, tag="st1")
                        nc.vector.tensor_reduce(st1[:, 0:1].unsqueeze(2),
                                                c1s[:].unsqueeze(1), axis=AX.X, op=OP.add)
                        nc.vector.tensor_reduce(st1[:, 1:2].unsqueeze(2),
                                                c1q[:].unsqueeze(1), axis=AX.X, op=OP.add)
                        nc.gpsimd.dma_start(st1_in[0:64, :], st1[:])
                        nc.gpsimd.collective_compute(
                            "AllReduce", OP.add, replica_groups=[list(range(NC_N))],
                            ins=[st1_in[:].opt()], outs=[st1_out[:].opt()])
                        sta = pSm.tile([64, 2], F32, tag="sta")
                        nc.sync.dma_start(sta[:], st1_out[0:64, :])
                        nc.vector.tensor_scalar(mut[:, 0:2], sta[:], 1.0 / NBN0,
                                                None, OP.mult)
                        musq = pSm.tile([64, 1], F32, tag="musq")
                        nc.vector.tensor_tensor(musq[:], mut[:, 0:1], mut[:, 0:1], op=OP.mult)
                        nc.vector.tensor_tensor(mut[:, 1:2], mut[:, 1:2], musq[:],
                                                op=OP.subtract)
                        nc.vector.tensor_scalar(mut[:, 1:2], mut[:, 1:2], EPS_BN, None, OP.add)
                        nc.scalar.sqrt(mut[:, 1:2], mut[:, 1:2])
                        nc.vector.reciprocal(mut[:, 1:2], mut[:, 1:2])
                        nc.vector.tensor_tensor(mut[:, 2:3], vecs[0:64, 125:126],
                                                mut[:, 1:2], op=OP.mult)
                        nc.vector.tensor_tensor(musq[:], mut[:, 0:1], mut[:, 2:3], op=OP.mult)
                        nc.vector.tensor_tensor(mut[:, 3:4], vecs[0:64, 126:127], musq[:],
                                                op=OP.subtract)
                        if dbg == "bn":
                            nc.sync.dma_start(dbg_d[:], mut[:])
                        if dbg == "y":
                            with tc.tile_pool(name="pDbgY", bufs=1) as pDbgY:
                                ydbg = pDbgY.tile([64, 16, F], F16, tag="ydbg")
                                nc.sync.dma_start(ydbg[:], y_dram[:, 0:16, :])
                                ydbf = pDbgY.tile([64, 16, F], F32, tag="ydbf")
                                nc.vector.tensor_copy(ydbf[:], ydbg[:])
                                nc.sync.dma_start(dbg_d[:], ydbf[:])


                    for q in range(Q if stop >= 5 else 0):
                        ypq = pH.tile([64, BQ, 130], F32R, tag="B")
                        nc.vector.memset(ypq[:, :, 0:1].bitcast(F32), 0.0)
                        nc.vector.memset(ypq[:, :, 129:130].bitcast(F32), 0.0)
                        for h in range(4):
                            yld = pX.tile([64, 32, F], F16, tag="x")
                            nc.sync.dma_start(
                                yld[:],
                                y_dram[:, q * BQ + 32 * h:q * BQ + 32 * (h + 1), :])
                            nc.scalar.activation(
                                ypq[:, 32 * h:32 * (h + 1), 1:129], yld[:],
                                AF.Relu, bias=mut[:, 3:4], scale=mut[:, 2:3])
                        if dbg == "yp" and q == 0:
                            nc.sync.dma_start(dbg_d[:], ypq[:, 0:16, 1:129].bitcast(F32))
                        zb = pH.tile([64, BQ, F], F32R, tag="A")
                        for i in range(32):
                            pz = pp.tile([F, 512], F32, tag="mm")
                            for k in range(3):
                                nc.tensor.matmul(
                                    pz[0:64, :].rearrange("p (b l) -> p b l", b=4),
                                    cwc[:, k, :], ypq[:, 4 * i:4 * i + 4, k:k + 128],
                                    start=(k == 0), stop=(k == 2))
                            if i % 2 == 0:
                                nc.vector.tensor_scalar(
                                    zb[:, 4 * i:4 * i + 4, :],
                                    pz[0:64, :].rearrange("p (b l) -> p b l", b=4),
                                    vecs[0:64, 124:125], None, OP.add)
                            else:
                                nc.scalar.activation(
                                    zb[:, 4 * i:4 * i + 4, :],
                                    pz[0:64, :].rearrange("p (b l) -> p b l", b=4),
                                    AF.Identity, bias=vecs[0:64, 124:125])
                        if dbg == "z" and q == 0:
                            nc.sync.dma_start(dbg_d[:], zb[:, 0:16, :].bitcast(F32))
                        n2q = pSm.tile([64, BQ], F32, tag="n2q")
                        for hh in range(8):
                            sqh = pX.tile([64, 16, F], F32, tag="x")
                            nc.scalar.activation(
                                sqh[:], zb[:, 16 * hh:16 * (hh + 1), :].bitcast(F32),
                                AF.Square)
                            nc.vector.tensor_reduce(
                                n2q[:, 16 * hh:16 * (hh + 1)].unsqueeze(2), sqh[:],
                                axis=AX.X, op=OP.add)
                        fq = pSm.tile([64, BQ], F32, tag="fq")
                        eq = pSm.tile([64, BQ], F32, tag="eq")
                        nc.scalar.sqrt(fq[:], n2q[:])
                        nc.scalar.activation(eq[:], fq[:], AF.Exp)
                        nc.vector.tensor_scalar(eq[:], eq[:], EPS_SQ, None, OP.add)
                        nc.vector.reciprocal(eq[:], eq[:])
                        nc.vector.tensor_scalar(eq[:], eq[:], -1.0, 1.0, OP.mult, OP.add)
                        nc.vector.tensor_scalar(fq[:], fq[:], EPS_SQ, None, OP.add)
                        nc.vector.reciprocal(fq[:], fq[:])
                        nc.vector.tensor_tensor(fq[:], fq[:], eq[:], op=OP.mult)
                        nc.vector.tensor_tensor(
                            zb[:], zb[:].bitcast(F32),
                            fq[:].unsqueeze(2).broadcast_to([64, BQ, F]), op=OP.mult)
                        for g in range(16):
                            psr = pr.tile([2, 8, BQ], F32, tag="rt")
                            for li in range(8):
                                l = 8 * g + li
                                nc.tensor.matmul(psr[:, li, :], rwt[:, l, :],
                                                 zb[:, :, l], start=True, stop=True)
                            s_sb = pStg.tile([2, 8, BQ], F32, tag="s_sb", bufs=2)
                            if g % 2 == 0:
                                nc.vector.tensor_copy(s_sb[:], psr[:])
                            else:
                                nc.scalar.copy(s_sb[:], psr[:])
                            nc.sync.dma_start(
                                s_loc[:, 8 * g:8 * g + 8, q * BQ:(q + 1) * BQ], s_sb[:])

            if dbg == "s":
                with tc.tile_pool(name="pDbg", bufs=1) as pDbg:
                    sdbg = pDbg.tile([2, F, 16], F32, tag="sdbg")
                    nc.sync.dma_start(sdbg[:], s_loc[:, :, 0:16])
                    nc.sync.dma_start(dbg_d[:], sdbg[:])

            if stop >= 6:
                nc.gpsimd.collective_compute(
                    "AllGather", OP.bypass, replica_groups=[list(range(NC_N))],
                    ins=[s_loc[:].opt()], outs=[s_all[:].opt()])

            if stop >= 7:
                with tc.tile_pool(name="pTail", bufs=1) as pT, \
                     tc.tile_pool(name="pStg2", bufs=3) as pStg2:
                    sf = pT.tile([F, NC_N, 2, BL], F32R, tag="TA")
                    nc.sync.dma_start(
                        sf[:], s_all[:].rearrange("(c r) l b -> c r l b", r=2)
                        .transpose([2, 0, 1, 3]).bitcast(F32R))
                    sfv = sf[:].rearrange("p a b c -> p (a b c)")
                    sqt = pT.tile([F, NT], F32R, tag="TB")
                    nc.vector.tensor_tensor(sqt[:], sfv.bitcast(F32), sfv.bitcast(F32),
                                            op=OP.mult)
                    n2t = pT.tile([8, NT], F32, tag="TC")
                    for i in range(16):
                        pn = pt.tile([8, 512], F32, tag="tp")
                        nc.tensor.matmul(pn[:], ones8[:], sqt[:, 512 * i:512 * (i + 1)],
                                         start=True, stop=True)
                        if i % 2 == 0:
                            nc.vector.tensor_copy(n2t[:, 512 * i:512 * (i + 1)], pn[:])
                        else:
                            nc.scalar.copy(n2t[:, 512 * i:512 * (i + 1)], pn[:])
                    n2r = pSm.tile([F, 64], F32, tag="n2r")
                    nc.sync.dma_start(n2r[:], n2t[0:1, :])
                    gr = pSm.tile([F, 64], F32, tag="gr")
                    er = pSm.tile([F, 64], F32, tag="er")
                    nc.scalar.sqrt(gr[:], n2r[:])
                    nc.scalar.activation(er[:], gr[:], AF.Exp)
                    nc.vector.tensor_scalar(er[:], er[:], EPS_SQ, None, OP.add)
                    nc.vector.reciprocal(er[:], er[:])
                    nc.vector.tensor_scalar(er[:], er[:], -1.0, 1.0, OP.mult, OP.add)
                    nc.vector.tensor_scalar(gr[:], gr[:], EPS_SQ, None, OP.add)
                    nc.vector.reciprocal(gr[:], gr[:])
                    nc.vector.tensor_tensor(gr[:], gr[:], er[:], op=OP.mult)
                    g1 = pT.tile([1, NT], F32R, tag="TC")
                    nc.sync.dma_start(g1[:], gr[:].bitcast(F32R))
                    for i in range(16):
                        pg = pt.tile([F, 512], F32, tag="tp")
                        nc.tensor.matmul(pg[:], vones[:], g1[:, 512 * i:512 * (i + 1)],
                                         start=True, stop=True)
                        nc.vector.tensor_tensor(sfv[:, 512 * i:512 * (i + 1)],
                                                sfv[:, 512 * i:512 * (i + 1)].bitcast(F32),
                                                pg[:], op=OP.mult)
                    mx = pT.tile([F, NC_N, BL], F32, tag="TD")
                    nc.vector.tensor_tensor(mx[:], sf[:, :, 0, :].bitcast(F32),
                                            sf[:, :, 1, :].bitcast(F32), op=OP.max)
                    y0 = pT.tile([F, NC_N, 2, BL], F32R, tag="TB")
                    for r in range(2):
                        nc.vector.tensor_tensor(y0[:, :, r, :], sf[:, :, r, :].bitcast(F32),
                                                mx[:], op=OP.add)
                    y0v = y0[:].rearrange("p a b c -> p (a b c)")
                    f1p = pT.tile([64, NT], F32, tag="TA")
                    b1s = pSm.tile([64, 16], F32, tag="b1s")
                    b1q = pSm.tile([64, 16], F32, tag="b1q")
                    for i in range(16):
                        pf = pp.tile([F, 512], F32, tag="mm")
                        nc.tensor.matmul(pf[0:64, :], fc1[:], y0v[:, 512 * i:512 * (i + 1)],
                                         start=True, stop=True)
                        nc.scalar.activation(f1p[:, 512 * i:512 * (i + 1)], pf[0:64, :],
                                             AF.Identity, accum_out=b1s[:, i:i + 1])
                        junk1 = pStg2.tile([64, 512], F32, tag="junk1")
                        nc.vector.scalar_tensor_tensor(
                            junk1[:], f1p[:, 512 * i:512 * (i + 1)], 1.0,
                            f1p[:, 512 * i:512 * (i + 1)], OP.mult, OP.mult,
                            accum_out=b1q[:, i:i + 1])
                    stp = pSm.tile([64, 4], F32, tag="stp")
                    nc.vector.tensor_reduce(stp[:, 0:2].unsqueeze(2),
                                            b1s[:].rearrange("p (c r) -> p r c", r=2),
                                            axis=AX.X, op=OP.add)
                    nc.vector.tensor_reduce(stp[:, 2:4].unsqueeze(2),
                                            b1q[:].rearrange("p (c r) -> p r c", r=2),
                                            axis=AX.X, op=OP.add)
                    ps4 = pt.tile([4, 1], F32, tag="tp")
                    nc.tensor.matmul(ps4[:], stp[:], vecs[0:64, 127:128], start=True, stop=True)
                    s4 = pSm.tile([4, 1], F32, tag="s4")
                    nc.vector.tensor_copy(s4[:], ps4[:])
                    nc.sync.dma_start(bscr[0:1, 0:4], s4[:])
                    r4 = pSm.tile([1, 4], F32, tag="r4")
                    nc.sync.dma_start(r4[:], bscr[0:1, 0:4])

                    def bn_rowstats(r4t, n_count, gcol, bcol, nm):
                        m = pSm.tile([1, 4], F32, tag="bnm" + nm)
                        nc.vector.tensor_scalar(m[:], r4t[:], 1.0 / n_count, None, OP.mult)
                        mu2 = pSm.tile([1, 2], F32, tag="bnu" + nm)
                        nc.vector.tensor_tensor(mu2[:], m[:, 0:2], m[:, 0:2], op=OP.mult)
                        nc.vector.tensor_tensor(m[:, 2:4], m[:, 2:4], mu2[:], op=OP.subtract)
                        nc.vector.tensor_scalar(m[:, 2:4], m[:, 2:4], EPS_BN, None, OP.add)
                        nc.scalar.sqrt(m[:, 2:4], m[:, 2:4])
                        nc.vector.reciprocal(m[:, 2:4], m[:, 2:4])
                        ac = pSm.tile([1, 4], F32, tag="bna" + nm)
                        nc.vector.tensor_tensor(ac[:, 0:2], vrow[:, gcol:gcol + 2],
                                                m[:, 2:4], op=OP.mult)
                        mua = pSm.tile([1, 2], F32, tag="bnw" + nm)
                        nc.vector.tensor_tensor(mua[:], m[:, 0:2], ac[:, 0:2],
                                                op=OP.mult)
                        nc.vector.tensor_tensor(ac[:, 2:4], vrow[:, bcol:bcol + 2],
                                                mua[:], op=OP.subtract)
                        return ac

                    ac1 = bn_rowstats(r4, NBN1, 0, 2, "1")
                    acb1 = pSm.tile([64, 4], F32, tag="acb1")
                    for j in range(4):
                        pb_ = pt.tile([64, 4], F32, tag="tp")
                        nc.tensor.matmul(pb_[:, 0:1], vrow[:, 8:72], ac1[:, j:j + 1],
                                         start=True, stop=True)
                        nc.vector.tensor_copy(acb1[:, j:j + 1], pb_[:, 0:1])
                    y1 = pT.tile([64, NT], F32R, tag="TC")
                    for i in range(16):
                        r = i % 2
                        nc.scalar.activation(y1[:, 512 * i:512 * (i + 1)],
                                             f1p[:, 512 * i:512 * (i + 1)], AF.Relu,
                                             scale=acb1[:, r:r + 1],
                                             bias=acb1[:, 2 + r:3 + r])
                    if dbg == "y1":
                        nc.sync.dma_start(dbg_d[:], y1[:, 0:16].bitcast(F32))
                    f2p = pT.tile([32, NT], F32, tag="TB")
                    b2s = pSm.tile([32, 16], F32, tag="b2s")
                    b2q = pSm.tile([32, 16], F32, tag="b2q")
                    for i in range(16):
                        pf = pp.tile([F, 512], F32, tag="mm")
                        nc.tensor.matmul(pf[0:32, :], fc2[:], y1[:, 512 * i:512 * (i + 1)],
                                         start=True, stop=True)
                        nc.scalar.activation(f2p[:, 512 * i:512 * (i + 1)], pf[0:32, :],
                                             AF.Identity, accum_out=b2s[:, i:i + 1])
                        junk2 = pStg2.tile([32, 512], F32, tag="junk2")
                        nc.vector.scalar_tensor_tensor(
                            junk2[:], f2p[:, 512 * i:512 * (i + 1)], 1.0,
                            f2p[:, 512 * i:512 * (i + 1)], OP.mult, OP.mult,
                            accum_out=b2q[:, i:i + 1])
                    stp2 = pSm.tile([32, 4], F32, tag="stp2")
                    nc.vector.tensor_reduce(stp2[:, 0:2].unsqueeze(2),
                                            b2s[:].rearrange("p (c r) -> p r c", r=2),
                                            axis=AX.X, op=OP.add)
                    nc.vector.tensor_reduce(stp2[:, 2:4].unsqueeze(2),
                                            b2q[:].rearrange("p (c r) -> p r c", r=2),
                                            axis=AX.X, op=OP.add)
                    ps42 = pt.tile([4, 1], F32, tag="tp")
                    nc.tensor.matmul(ps42[:], stp2[:], vecs[0:32, 127:128],
                                     start=True, stop=True)
                    s42 = pSm.tile([4, 1], F32, tag="s42")
                    nc.vector.tensor_copy(s42[:], ps42[:])
                    nc.sync.dma_start(bscr[0:1, 4:8], s42[:])
                    r42 = pSm.tile([1, 4], F32, tag="r42")
                    nc.sync.dma_start(r42[:], bscr[0:1, 4:8])
                    ac2 = bn_rowstats(r42, NBN2, 4, 6, "2")
                    acb2 = pSm.tile([32, 4], F32, tag="acb2")
                    for j in range(4):
                        pb_ = pt.tile([64, 4], F32, tag="tp")
                        nc.tensor.matmul(pb_[0:32, 0:1], vrow[:, 8:40], ac2[:, j:j + 1],
                                         start=True, stop=True)
                        nc.vector.tensor_copy(acb2[:, j:j + 1], pb_[0:32, 0:1])
                    y2 = pT.tile([32, NT], F32R, tag="TA")
                    for i in range(16):
                        r = i % 2
                        nc.scalar.activation(y2[:, 512 * i:512 * (i + 1)],
                                             f2p[:, 512 * i:512 * (i + 1)], AF.Relu,
                                             scale=acb2[:, r:r + 1],
                                             bias=acb2[:, 2 + r:3 + r])
                    y3 = pT.tile([1, NT], F32, tag="TC")
                    for i in range(8):
                        pf3 = pr.tile([1, 1024], F32, tag="rt")
                        for j in range(2):
                            nc.tensor.matmul(
                                pf3[:, 512 * j:512 * (j + 1)], fc3[:],
                                y2[:, 1024 * i + 512 * j:1024 * i + 512 * (j + 1)],
                                start=True, stop=True)
                        nc.scalar.activation(y3[:, 1024 * i:1024 * (i + 1)], pf3[:],
                                             AF.Sigmoid)
                    o1 = pT.tile([1, B], F32, tag="TD")
                    y3v = y3[:].rearrange("p (c r b) -> p c r b", r=2, b=BL)
                    nc.vector.tensor_tensor(o1[:].rearrange("p (c b) -> p c b", b=BL),
                                            y3v[:, :, 0, :], y3v[:, :, 1, :], op=OP.max)
                    nc.sync.dma_start(out_d[:].unsqueeze(0), o1[:])


    fix_instruction_waits(nc)
    return nc


def host_prep(inputs):
    x = np.asarray(inputs["x"])
    emb = np.asarray(inputs["emb"], np.float32)
    W3 = np.asarray(inputs["W3"], np.float32)
    b3 = np.asarray(inputs["b3"], np.float32)
    W5 = np.asarray(inputs["W5"], np.float32)
    b5 = np.asarray(inputs["b5"], np.float32)
    W7 = np.asarray(inputs["W7"], np.float32)
    b7 = np.asarray(inputs["b7"], np.float32)
    conv1_w = np.asarray(inputs["conv1_w"], np.float32)
    conv1_b = np.asarray(inputs["conv1_b"], np.float32)
    bn_g = np.asarray(inputs["bn_g"], np.float32)
    bn_b = np.asarray(inputs["bn_b"], np.float32)
    caps_w = np.asarray(inputs["caps_w"], np.float32)
    caps_b = np.asarray(inputs["caps_b"], np.float32)
    route_W = np.asarray(inputs["route_W"], np.float32)
    route_b = np.asarray(inputs["route_b"], np.float32)
    fc1_w = np.asarray(inputs["fc1_w"], np.float32)
    bn1_g = np.asarray(inputs["bn1_g"], np.float32)
    bn1_b = np.asarray(inputs["bn1_b"], np.float32)
    fc2_w = np.asarray(inputs["fc2_w"], np.float32)
    bn2_g = np.asarray(inputs["bn2_g"], np.float32)
    bn2_b = np.asarray(inputs["bn2_b"], np.float32)
    fc3_w = np.asarray(inputs["fc3_w"], np.float32)

    w3h = np.einsum("vf,sjfo->svjo", emb, W3.reshape(S, 3, F, F)).astype(np.float32)
    w5h = np.ascontiguousarray(W5.reshape(S, 5, F, F).transpose(0, 2, 1, 3))
    w7h = np.ascontiguousarray(W7.reshape(S, 7, F, F).transpose(0, 2, 1, 3))
    cw1 = np.zeros((128, 64), np.float32)
    for k in range(3):
        cw1[k * S:(k + 1) * S, :] = conv1_w[:, :, k].T
    cwc = np.zeros((64, 3, 64), np.float32)
    for g in range(2):
        cwc[32 * g:32 * (g + 1), :, 32 * g:32 * (g + 1)] = np.transpose(
            caps_w[32 * g:32 * (g + 1)], (1, 2, 0))
    rwp = route_W * (1.0 + route_b)
    rwt = np.ascontiguousarray(np.transpose(rwp, (1, 2, 0)))
    vecs = np.zeros((F, 128), np.float32)
    vecs[:, 127] = 1.0
    vecs[:, 0:41] = b3.T
    vecs[:, 41:82] = b5.T
    vecs[:, 82:123] = b7.T
    vecs[:, 123] = np.concatenate([conv1_b, conv1_b])
    vecs[0:64, 124] = caps_b
    vecs[64:128, 124] = caps_b
    vecs[0:64, 125] = bn_g
    vecs[0:64, 126] = bn_b
    vrow = np.zeros((1, 80), np.float32)
    vrow[0, 0:8] = np.concatenate([bn1_g, bn1_b, bn2_g, bn2_b])
    vrow[0, 8:72] = 1.0

    common = dict(
        w3=w3h, w5=w5h, w7=w7h, cw1=cw1, cwc=cwc, rwt=rwt,
        fc1=np.ascontiguousarray(fc1_w), fc2=np.ascontiguousarray(fc2_w),
        fc3=np.ascontiguousarray(fc3_w),
        vecs=vecs, vrow=vrow,
        vones=np.ones((1, F), np.float32),
        ones8=np.ones((F, 8), np.float32),
        ident=np.eye(F, dtype=np.float32),
    )
    in_maps = []
    for c in range(NC_N):
        xs = x[c * BL:(c + 1) * BL]
        oh = np.zeros((4, S + 2, BL), np.float32)
        oh[:, 1:S + 1, :] = (xs.T[None, :, :] == np.arange(4)[:, None, None])
        m = dict(common)
        m["oh"] = oh
        in_maps.append(m)
    return in_maps


_CACHE = {}


def kernel(**inputs):
    import os
    dbg = os.environ.get("KDBG") or None
    stop = int(os.environ.get("KSTOP", "7"))
    key = ("nc", dbg, stop)
    if key not in _CACHE:
        _CACHE[key] = build(dbg, stop)
    nc = _CACHE[key]
    in_maps = host_prep(inputs)
    res = run_bass_kernel_spmd(nc, in_maps, list(range(NC_N)))
    out = np.asarray(res.results[0]["out"], np.float32).reshape(B, 1)
    if dbg:
        kernel.dbg = [np.asarray(r["dbg"]) for r in res.results]
    return out


if __name__ == "__main__":
    import tempfile
    from concourse.bass_utils import compile_bass_kernel
    nc = build(None)
    d = tempfile.mkdtemp()
    compile_bass_kernel(nc, d)
    print("compile OK", d)

